# revision 1
# baseline (speedup 1.0000x reference)
"""Trainium2 Bass kernel v2 for CharacterBERT CharCNN.

Per-core structure (512 tokens):
  Host: x_img [115, 512, 52] bf16 skewed conv image. Row 16*dw+c at
    (t, j) holds emb[ids[t, j-1+dw], c]; j=0 is an all-zero column so
    the max-pool folds relu. Rows 112/113 are w2/w3 invalid-position
    indicators (paired with -30000 in the weight), row 114 is the bias
    row. Conv weights are pre-scaled by S_T so conv output lands in
    fp8-friendly range.
  Conv: per 128-filter chunk, 3D-AP matmuls [115, 8tok, jn] into PSUM
    (jn = valid positions + zero col, trimmed per filter width),
    24 tokens per 3-bank PSUM tile.
  Maxpool+relu: route A = segmented tensor_reduce(max) from PSUM on
    DVE; route B = ACT copy PSUM->SBUF stage, then DVE overlap-fold max
    tree (all-bf16 SBUF ops get the 2x DVE mode). T0/T1 bf16 feed the
    highway combine; fp8 shadows T0q/T1q feed the DoubleRow matmuls
    (writing the combine path in fp8 costs ~1.4e-2 through the gated
    identity path -- measured, so the shadows are necessary).
  Highway x2: fp8 DoubleRow matmuls (K=256/pass), sigmoid/relu on ACT
    straight from PSUM with folded descales, combine on DVE.
  Proj: bf16 matmuls, Identity+bias+descale on ACT, fp32 out.
Emission interleaves conv(half 1) with highway+proj(half 0) to overlap
PE with the vector engines.
"""

import numpy as np
import ml_dtypes
from contextlib import ExitStack

import concourse.bass as bass
import concourse.mybir as mybir
import concourse.tile as tile
from concourse import bacc
from concourse.bass_utils import run_bass_kernel_spmd

BF16 = mybir.dt.bfloat16
F32 = mybir.dt.float32
FP8 = mybir.dt.float8e4
AF = mybir.ActivationFunctionType
AX = mybir.AxisListType
ALU = mybir.AluOpType
DR = mybir.MatmulPerfMode.DoubleRow

B, S, MAX_CHARS = 8, 512, 50
EMB = 16
VOCAB = 264
TOTAL_F = 2048
HIDDEN = 768
FILTERS = [(1, 32), (2, 32), (3, 64), (4, 128), (5, 256), (6, 512), (7, 1024)]
NCORES = 8

T = 512                   # tokens per core
JW = 52                   # j columns per token in x_img (j=0 zero col)
KX = 115                  # x rows: 112 patch + 2 masks + bias
NKF = 16
CHUNK_W = [3, 4, 5, 5, 6, 6, 6, 6, 7, 7, 7, 7, 7, 7, 7, 7]
CHUNK_JN = [51] + [52 - w for w in CHUNK_W[1:]]
GB = 3                    # psum banks per conv group
CHUNK_GT = [512 // jn for jn in CHUNK_JN]   # tokens per conv matmul/bank
NEG = -30000.0
S_T = 1024.0
S_W = 512.0
NOH = 32
NOP = 6
HALVES = ((0, 264), (264, 248))
POOL_EVERY = 3            # every POOL_EVERY-th conv group -> ACT+Pool
MASKN = 1224              # >= GT*51

_BF = ml_dtypes.bfloat16
_F8 = ml_dtypes.float8_e4m3


def _groups(t0, ntok, gt):
    gtok = gt * GB
    t = t0
    while t < t0 + ntok:
        n = min(gtok, t0 + ntok - t)
        gb = (n + gt - 1) // gt
        yield t, gb, n - (gb - 1) * gt
        t += n


def build_program(ExitStackCls=ExitStack, pool_every=POOL_EVERY, loop_n=0):
    """loop_n > 0 wraps the whole body in a hardware loop (timing only)."""
    nc = bacc.Bacc("TRN2", target_bir_lowering=False, debug=False)

    d_x = nc.dram_tensor("x", [KX, T * JW], BF16, kind="ExternalInput").ap()
    d_wall = nc.dram_tensor("wall", [KX, TOTAL_F], BF16, kind="ExternalInput").ap()
    d_hw0 = nc.dram_tensor("hw0w", [NOH, 128, TOTAL_F], FP8, kind="ExternalInput").ap()
    d_hw1 = nc.dram_tensor("hw1w", [NOH, 128, TOTAL_F], FP8, kind="ExternalInput").ap()
    d_prj = nc.dram_tensor("prjw", [NOP, 128, TOTAL_F], BF16, kind="ExternalInput").ap()
    d_hwb = nc.dram_tensor("hwb", [128, 64], F32, kind="ExternalInput").ap()
    d_prjb = nc.dram_tensor("prjb", [128, NOP], F32, kind="ExternalInput").ap()
    d_out = nc.dram_tensor("out", [NOP, 128, T], F32, kind="ExternalOutput").ap()
    d_hw = [d_hw0, d_hw1]

    with tile.TileContext(nc) as tc, ExitStackCls() as ctx:
        loop = tc.For_i(0, loop_n) if loop_n else None
        if loop is not None:
            ctx.enter_context(loop)
        const = ctx.enter_context(tc.tile_pool(name="const", bufs=1))
        xp = ctx.enter_context(tc.tile_pool(name="xp", bufs=1))
        tp = ctx.enter_context(tc.tile_pool(name="tp", bufs=1))
        stg = ctx.enter_context(tc.tile_pool(name="stg", bufs=2))
        hww_p = ctx.enter_context(tc.tile_pool(name="hww", bufs=4))
        hwtmp = ctx.enter_context(tc.tile_pool(name="hwtmp", bufs=3))
        outp = ctx.enter_context(tc.tile_pool(name="outp", bufs=2))
        convps = ctx.enter_context(tc.tile_pool(name="convps", bufs=2, space="PSUM"))
        hwps = ctx.enter_context(tc.tile_pool(name="hwps", bufs=2, space="PSUM"))

        wall_t = const.tile([KX, TOTAL_F], BF16)
        nc.sync.dma_start(wall_t[:], d_wall[:])
        hwb_t = const.tile([128, 64], F32)
        nc.sync.dma_start(hwb_t[:], d_hwb[:])
        prjb_t = const.tile([128, NOP], F32)
        nc.sync.dma_start(prjb_t[:], d_prjb[:])

        x_t = xp.tile([KX, T, JW], BF16)
        for t0 in range(0, T, 128):
            nc.sync.dma_start(
                x_t[:, t0:t0 + 128, :],
                d_x[:, t0 * JW:(t0 + 128) * JW].rearrange(
                    "p (t j) -> p t j", j=JW))

        t0_t = tp.tile([128, NKF, T], BF16, tag="t0")
        t1_t = tp.tile([128, NKF, T], BF16, tag="t1")
        t2_t = tp.tile([128, NKF, T], BF16, tag="t2")
        t0q_t = tp.tile([128, NKF, T], FP8, tag="t0q")
        t1q_t = tp.tile([128, NKF, T], FP8, tag="t1q")
        t_tiles = [t0_t, t1_t, t2_t]
        tq_tiles = [t0q_t, t1q_t]

        def conv_group_a(k, tok0, gb, gt_last):
            """Route A: conv matmuls + direct DVE segmented reduce."""
            jn = CHUNK_JN[k]
            gt0 = CHUNK_GT[k]
            gtok = (gb - 1) * gt0 + gt_last
            lhsT = wall_t[:, 128 * k:128 * (k + 1)]
            ps = convps.tile([128, GB, 512], F32)
            for b in range(gb):
                gt = gt0 if b < gb - 1 else gt_last
                nc.tensor.matmul(
                    ps[:, b, :gt * jn], lhsT=lhsT,
                    rhs=x_t[:, tok0 + b * gt0:tok0 + b * gt0 + gt, :jn],
                    start=True, stop=True)
            if gt_last == gt0:
                nc.vector.tensor_reduce(
                    t0_t[:, k, tok0:tok0 + gtok],
                    ps[:, 0:gb, :gt0 * jn].rearrange(
                        "p b (t j) -> p b t j", j=jn),
                    axis=AX.X, op=ALU.max)
            else:
                if gb > 1:
                    nc.vector.tensor_reduce(
                        t0_t[:, k, tok0:tok0 + (gb - 1) * gt0],
                        ps[:, 0:gb - 1, :gt0 * jn].rearrange(
                            "p b (t j) -> p b t j", j=jn),
                        axis=AX.X, op=ALU.max)
                nc.vector.tensor_reduce(
                    t0_t[:, k, tok0 + (gb - 1) * gt0:tok0 + gtok],
                    ps[:, gb - 1:gb, :gt_last * jn].rearrange(
                        "p b (t j) -> p b t j", j=jn),
                    axis=AX.X, op=ALU.max)

        def stage_fill_b(k, tb0, ngr):
            """Route B fill: conv matmuls + ACT copies PSUM->stage."""
            jn = CHUNK_JN[k]
            gt0 = CHUNK_GT[k]
            gtok = gt0 * GB
            lhsT = wall_t[:, 128 * k:128 * (k + 1)]
            stage = stg.tile([128, 136, JW], BF16, tag="stage")
            for g in range(ngr):
                tok0 = tb0 + g * gtok
                ps = convps.tile([128, GB, 512], F32)
                for b in range(GB):
                    nc.tensor.matmul(
                        ps[:, b, :gt0 * jn], lhsT=lhsT,
                        rhs=x_t[:, tok0 + b * gt0:tok0 + b * gt0 + gt0, :jn],
                        start=True, stop=True)
                nc.scalar.copy(
                    stage[:, g * gtok:(g + 1) * gtok, 0:jn].rearrange(
                        "p (b t) j -> p b t j", b=GB),
                    ps[:, 0:GB, :gt0 * jn].rearrange(
                        "p b (t j) -> p b t j", j=jn))
            return (stage, k, tb0, ngr)

        def stage_tree_b(st):
            """Route B reduce: DVE overlap-fold tree, writes T0."""
            stage, k, tb0, ngr = st
            jn = CHUNK_JN[k]
            ntok = ngr * CHUNK_GT[k] * GB
            s = jn
            while s > 2:
                h = (s + 1) // 2
                nc.vector.tensor_tensor(
                    out=stage[:, :ntok, 0:h],
                    in0=stage[:, :ntok, 0:h],
                    in1=stage[:, :ntok, s - h:s],
                    op=ALU.max)
                s = h
            nc.vector.tensor_tensor(
                out=t0_t[:, k, tb0:tb0 + ntok].rearrange("p (t o) -> p t o", o=1),
                in0=stage[:, :ntok, 0:1],
                in1=stage[:, :ntok, 1:2],
                op=ALU.max)

        pending = []
        quad_ctr = [0]

        def flush_pending():
            while pending:
                kind, payload = pending.pop(0)
                if kind == "tree":
                    stage_tree_b(payload)
                else:
                    kq, t0q, ntokq = payload
                    nc.vector.tensor_copy(
                        t0q_t[:, kq, t0q:t0q + ntokq],
                        t0_t[:, kq, t0q:t0q + ntokq])

        def conv_chunk(k, h):
            """Emit conv for chunk k of half h. Route-B trees and the fp8
            shadow copy are deferred to the next conv_chunk call (pending
            list) so the strict-FIFO DVE queue isn't blocked on ACT."""
            t0, ntok = HALVES[h]
            gt0 = CHUNK_GT[k]
            groups = list(_groups(t0, ntok, gt0))
            nf = sum(1 for g in groups if g[1] == GB and g[2] == gt0)
            # stages of 4 full groups each (<= 136 tokens per stage tile)
            nstg = (nf // 4) if pool_every else 0
            na_head = nf - 4 * nstg
            gi = 0
            for _ in range(na_head):
                tok0, gb, gt_last = groups[gi]
                conv_group_a(k, tok0, gb, gt_last)
                gi += 1
            fills = []
            amod = 5 if h == 0 else 0   # h0: 1-in-5 quads direct-DVE
            for _ in range(nstg):
                quad_ctr[0] += 1
                if amod and quad_ctr[0] % amod == 0:
                    for _ in range(4):
                        tok0, gb, gt_last = groups[gi]
                        conv_group_a(k, tok0, gb, gt_last)
                        gi += 1
                else:
                    fills.append(stage_fill_b(k, groups[gi][0], 4))
                    gi += 4
            while gi < len(groups):
                tok0, gb, gt_last = groups[gi]
                conv_group_a(k, tok0, gb, gt_last)
                gi += 1
            flush_pending()
            for st in fills:
                pending.append(("tree", st))
            pending.append(("q", (k, t0, ntok)))

        hw_pending = []

        def hw_mm(layer, j, h):
            """Matmul part of a highway unit; act/combine deferred one
            unit to avoid ACT/DVE head-of-line stalls (strict FIFO)."""
            t0, ntok = HALVES[h]
            tq_in = tq_tiles[layer]
            w_nl = hww_p.tile([128, TOTAL_F], FP8, tag="wnl")
            nc.sync.dma_start(w_nl[:], d_hw[layer][j, :, :])
            w_g = hww_p.tile([128, TOTAL_F], FP8, tag="wg")
            nc.sync.dma_start(w_g[:], d_hw[layer][16 + j, :, :])
            psts = []
            for w_t in (w_nl, w_g):
                pst = hwps.tile([128, 512], F32, tag="hwps")
                for kc in range(8):
                    nc.tensor.matmul(
                        pst[:, :ntok],
                        lhsT=w_t[:, 256 * kc:256 * (kc + 1)].rearrange(
                            "p (two m) -> p two m", two=2),
                        rhs=tq_in[:, 2 * kc:2 * kc + 2, t0:t0 + ntok],
                        start=(kc == 0), stop=(kc == 7),
                        perf_mode=DR,
                    )
                psts.append(pst)
            return (layer, j, h, psts)

        def hw_act(state):
            layer, j, h, (ps_nl, ps_g) = state
            t0, ntok = HALVES[h]
            t_in = t_tiles[layer]
            rl = hwtmp.tile([128, 512], BF16, tag="rl")
            sg = hwtmp.tile([128, 512], BF16, tag="sg")
            nc.scalar.activation(
                rl[:, :ntok], ps_nl[:, :ntok], AF.Relu,
                bias=hwb_t[:, layer * 32 + j:layer * 32 + j + 1],
                scale=1.0 / S_W)
            nc.scalar.activation(
                sg[:, :ntok], ps_g[:, :ntok], AF.Sigmoid,
                bias=hwb_t[:, layer * 32 + 16 + j:layer * 32 + 16 + j + 1],
                scale=1.0 / (S_T * S_W))
            dd = hwtmp.tile([128, 512], BF16, tag="dd")
            nc.vector.tensor_sub(dd[:, :ntok], t_in[:, j, t0:t0 + ntok],
                                 rl[:, :ntok])
            ee = hwtmp.tile([128, 512], BF16, tag="ee")
            nc.vector.tensor_mul(ee[:, :ntok], sg[:, :ntok], dd[:, :ntok])
            nc.vector.tensor_add(t_tiles[layer + 1][:, j, t0:t0 + ntok],
                                 ee[:, :ntok], rl[:, :ntok])
            if layer == 0:
                nc.vector.tensor_copy(t1q_t[:, j, t0:t0 + ntok],
                                      t1_t[:, j, t0:t0 + ntok])

        def hw_chunk(layer, j, h):
            while hw_pending:
                hw_act(hw_pending.pop(0))
            hw_pending.append(hw_mm(layer, j, h))

        def hw_flush():
            while hw_pending:
                hw_act(hw_pending.pop(0))

        def proj_chunk(o, h):
            t0, ntok = HALVES[h]
            w_p = hww_p.tile([128, TOTAL_F], BF16, tag="wp")
            nc.sync.dma_start(w_p[:], d_prj[o, :, :])
            pst = hwps.tile([128, 512], F32, tag="hwps")
            for kc in range(16):
                nc.tensor.matmul(
                    pst[:, :ntok],
                    lhsT=w_p[:, 128 * kc:128 * (kc + 1)],
                    rhs=t_tiles[2][:, kc, t0:t0 + ntok],
                    start=(kc == 0), stop=(kc == 15),
                )
            ot = outp.tile([128, 512], F32, tag="ot")
            nc.scalar.activation(ot[:, :ntok], pst[:, :ntok],
                                 AF.Identity, bias=prjb_t[:, o:o + 1],
                                 scale=1.0 / S_T)
            nc.sync.dma_start(out=d_out[o, :, t0:t0 + ntok], in_=ot[:, :ntok])

        # ---- emission schedule ----
        for k in range(NKF):
            conv_chunk(k, 0)
        flush_pending()
        # conv(h1) interleaved with chain(h0)
        chain0 = ([("hw", 0, j, 0) for j in range(16)]
                  + [("hw", 1, j, 0) for j in range(16)]
                  + [("proj", o, 0) for o in range(NOP)])
        conv1 = [("conv", k, 1) for k in range(NKF)]
        sched, ci, hi = [], 0, 0
        pat = [2, 2, 3]
        while ci < len(conv1) or hi < len(chain0):
            if ci < len(conv1):
                sched.append(conv1[ci]); ci += 1
            for _ in range(pat[ci % 3] if ci < len(conv1) else 99):
                if hi < len(chain0):
                    sched.append(chain0[hi]); hi += 1
        for item in sched:
            if item[0] == "conv":
                conv_chunk(item[1], item[2])
            elif item[0] == "hw":
                hw_chunk(item[1], item[2], item[3])
            else:
                hw_flush()
                proj_chunk(item[1], item[2])
        hw_flush()
        flush_pending()
        # chain(h1)
        for layer in range(2):
            for j in range(16):
                hw_chunk(layer, j, 1)
            hw_flush()
        for o in range(NOP):
            proj_chunk(o, 1)

    nc.compile()
    return nc


# ---------------- host-side preparation ----------------

def prep_shared(char_emb, conv_ws, conv_bs, hw_ws, hw_bs, proj_w, proj_b):
    out = {}
    wall = np.zeros((KX, TOTAL_F), dtype=np.float32)
    fbase = 0
    for (w, nf), cw, cb in zip(FILTERS, conv_ws, conv_bs):
        cw = np.asarray(cw, np.float32)
        for dw in range(w):
            wall[16 * dw:16 * (dw + 1), fbase:fbase + nf] = S_T * cw[:, :, dw].T
        if w == 2:
            wall[112, fbase:fbase + nf] = NEG
        if w == 3:
            wall[113, fbase:fbase + nf] = NEG
        wall[114, fbase:fbase + nf] = S_T * np.asarray(cb, np.float32)
        fbase += nf
    out["wall"] = wall.astype(_BF)

    def repack(wm, no, scale, dt):
        wm = np.asarray(wm, np.float32) * scale
        kk = wm.shape[0] // 128
        r = (wm.reshape(kk, 128, no, 128).transpose(2, 1, 0, 3)
             .reshape(no, 128, kk * 128))
        if dt is _F8:
            r = np.clip(r, -230, 230)
        return r.astype(dt)

    out["hw0w"] = repack(hw_ws[0], NOH, S_W, _F8)
    out["hw1w"] = repack(hw_ws[1], NOH, S_W, _F8)
    out["prjw"] = repack(proj_w, NOP, 1.0, _BF)

    hwb = np.zeros((128, 64), dtype=np.float32)
    for layer in range(2):
        hb = np.asarray(hw_bs[layer], np.float32)
        for j in range(16):
            hwb[:, layer * 32 + j] = S_T * hb[128 * j:128 * (j + 1)]
            hwb[:, layer * 32 + 16 + j] = hb[TOTAL_F + 128 * j:TOTAL_F + 128 * (j + 1)]
    out["hwb"] = hwb
    out["prjb"] = np.asarray(proj_b, np.float32).reshape(NOP, 128).T.copy()
    return out


def prep_x(ids_core, char_emb):
    """ids_core [T, 50] -> x_img [115, T*52] bf16."""
    emb = np.asarray(char_emb, np.float32)
    embz = np.concatenate([emb, np.zeros((1, EMB), np.float32)], 0)
    ids = np.asarray(ids_core, np.int64)
    ids_pad = np.full((T, MAX_CHARS + 7), VOCAB, np.int64)
    ids_pad[:, :MAX_CHARS] = ids
    x = np.zeros((KX, T, JW), dtype=np.float32)
    for dw in range(7):
        g = embz[ids_pad[:, dw:dw + 51]]          # [T, 51, 16]
        x[16 * dw:16 * (dw + 1), :, 1:] = g.transpose(2, 0, 1)
    x[112, :, 50:] = 1.0   # w2 invalid j (mask row)
    x[113, :, 49:] = 1.0   # w3 invalid j
    x[114, :, 1:] = 1.0    # bias row
    return x.reshape(KX, T * JW).astype(_BF)


_CACHED_NC = None


def _get_nc():
    global _CACHED_NC
    if _CACHED_NC is None:
        _CACHED_NC = build_program()
    return _CACHED_NC


def make_in_maps(inputs):
    ii = {k: np.asarray(v) for k, v in inputs.items()}
    conv_ws = [ii[f"conv_w{i}"] for i in range(7)]
    conv_bs = [ii[f"conv_b{i}"] for i in range(7)]
    shared = prep_shared(
        ii["char_emb"], conv_ws, conv_bs,
        [ii["hw_w0"], ii["hw_w1"]], [ii["hw_b0"], ii["hw_b1"]],
        ii["proj_w"], ii["proj_b"],
    )
    ids = ii["input_ids"].reshape(-1, MAX_CHARS)
    in_maps = []
    for c in range(NCORES):
        m = dict(shared)
        m["x"] = prep_x(ids[c * T:(c + 1) * T], ii["char_emb"])
        in_maps.append(m)
    return in_maps


def run(inputs, trace=False, **kw):
    in_maps = make_in_maps(inputs)
    res = run_bass_kernel_spmd(_get_nc(), in_maps, list(range(NCORES)),
                               trace=trace, **kw)
    outs = []
    for c in range(NCORES):
        o = np.asarray(res.results[c]["out"])
        outs.append(o.reshape(HIDDEN, T).T)
    full = np.stack(outs, axis=0).reshape(B, S, HIDDEN).astype(np.float32)
    return full, res


def kernel(**inputs):
    return run(inputs)[0]


if __name__ == "__main__":
    build_program()
    print("build ok")

